# revision 18
# baseline (speedup 1.0000x reference)
"""Trainium2 Bass kernel for AutomatonPELayer (path-graph GNN solve).

Reference computes ``pe = reshape(solve(I - kron(adj, T), tile(p, n)), (n, k))``
with ``adj`` the path-graph adjacency on n=256 nodes and T a 16x16 matrix with
||T|| = 0.45.

Math: the path graph has the analytic eigendecomposition ``adj = V diag(lam)
V^T`` (DST-I), so with mu_j = lam_j / 2 and S = 2T,

    X = C @ G^T,   C[i, m] = sum_j V[i,j] * s_j * mu_j^m   (host constant),
    G^T[m, :]     = (S^m p)^T                              (device Krylov block),

where s_j = sum_i V[i,j] and the Neumann series is truncated at M = 128 terms
(spectral radius of mu*S is <= 0.9, truncation ~0.9^128 ~ 1e-6 relative).

Device work per core: build the M=128 Krylov columns by 7 doubling levels
(G_{2r} = [G_r, S^r G_r] plus squaring chains for S^r / (S^T)^r), transpose,
then one [128,32]^T @ [128,16] matmul against this core's 32-row slice of C.
Core c returns output rows [32c, 32c+32); the host concatenates.

All matmuls run in float32r (single-pass fp32) to halve PE instruction count;
all small inputs ship in one [16,49] DMA whose last column doubles as G's
first column.
"""

import numpy as np

N = 256          # sentence length (path-graph nodes)
K = 16           # automaton state dim
M = 128          # Neumann/Krylov truncation order
LEVELS = 7       # doublings: block sizes 1,2,4,...,64 -> 128 columns
NUM_CORES = 8
ROWS_PER_CORE = N // NUM_CORES

# column layout of the packed small input: [Q0 | R0 | eye | p]
_COL_Q0 = 0
_COL_R0 = K
_COL_EYE = 2 * K
_COL_P = 3 * K
_SMALL_COLS = 3 * K + 1       # 49
_G0 = _COL_P                  # G occupies columns [_G0, _G0 + M/2) of the big tile
_BIG_COLS = _G0 + M // 2      # 112; G's top half is produced pre-transposed


def _host_constants():
    """C[i, m] = sum_j V[i,j] * s_j * mu_j^m, in float64, cast to f32."""
    j = np.arange(1, N + 1)
    theta = j * np.pi / (N + 1)
    V = np.sqrt(2.0 / (N + 1)) * np.sin(np.outer(np.arange(1, N + 1), theta))
    s = V.sum(axis=0)
    mu = np.cos(theta)
    vand = mu[None, :] ** np.arange(M)[:, None]        # [M, j]
    C = (V * s[None, :]) @ vand.T                      # [N(i), M]
    return np.ascontiguousarray(C.astype(np.float32))


_CACHE = {}


def _patch_walrus_flags():
    """Cap walrus's semaphore allocation so the NEFF epilogue resets ~64
    semaphores instead of all 256 (the per-sem resets dominate the kernel
    tail for a kernel this small)."""
    if _CACHE.get("walrus_patched"):
        return
    import concourse.bass_utils as bu

    orig = bu.bir_verify_and_optimise

    def patched(tmpdir, inp="bir.json", outp="file.neff", arch=None, *, dve_root=None):
        orig_run = bu.run_command

        def run_with_flag(cmd, **kw):
            if cmd and "walrus_driver" in str(cmd[0]):
                cmd = list(cmd) + ["--max-sem-num=64"]
            return orig_run(cmd, **kw)

        bu.run_command = run_with_flag
        try:
            return orig(tmpdir, inp, outp, arch, dve_root=dve_root)
        finally:
            bu.run_command = orig_run

    bu.bir_verify_and_optimise = patched
    _CACHE["walrus_patched"] = True


def _build_bass():
    import concourse.mybir as mybir
    import concourse.tile as tile
    from concourse import bacc

    nc = bacc.Bacc(
        "TRN2",
        target_bir_lowering=False,
        debug=False,
        enable_asserts=False,
        num_devices=NUM_CORES,
    )
    dt = mybir.dt.float32
    f32r = mybir.dt.float32r

    def r_(ap):
        return ap.bitcast(f32r)

    small = nc.dram_tensor("small", [K, _SMALL_COLS], dt, kind="ExternalInput").ap()
    ct = nc.dram_tensor("ct", [M, ROWS_PER_CORE], dt, kind="ExternalInput").ap()
    out = nc.dram_tensor("out", [ROWS_PER_CORE, K], dt, kind="ExternalOutput").ap()

    H = M // 2  # 64
    with tile.TileContext(nc) as tc:
        with (
            tc.tile_pool(name="const", bufs=1) as cpool,
            tc.tile_pool(name="pg", bufs=2, space="PSUM") as pg_pool,
            tc.tile_pool(name="pq", bufs=2, space="PSUM") as pq_pool,
            tc.tile_pool(name="pgt", bufs=2, space="PSUM") as pgt_pool,
            tc.tile_pool(name="px", bufs=1, space="PSUM") as px_pool,
        ):
            big = cpool.tile([K, _BIG_COLS], dt, tag="big")
            gt = cpool.tile([M, K], dt, tag="gt")
            ct_t = cpool.tile([M, ROWS_PER_CORE], dt, tag="ct")
            # ping-pong [32,32] tiles so the DVE 32-block transpose that
            # derives R_{l+1} = Q_{l+1}^T reads fully initialized data
            qt = [cpool.tile([32, 32], dt, tag=f"q{i}", name=f"qt{i}")
                  for i in range(2)]
            rt = [cpool.tile([32, 32], dt, tag=f"r{i}", name=f"rt{i}")
                  for i in range(2)]

            # one DMA for Q0/R0/eye/p (p lands at column _COL_P = G's col 0).
            # Scalar exits the NEFF preamble ~1.2us before Sync, so it issues
            # this critical DMA.
            nc.scalar.dma_start(out=big[:, 0:_SMALL_COLS], in_=small[:, :])
            # ct is only needed by the last matmuls; fetch in parallel
            nc.sync.dma_start(out=ct_t[:], in_=ct[:])
            nc.gpsimd.memset(qt[0][:], 0.0)
            nc.gpsimd.memset(qt[1][:], 0.0)

            def g_cols(lo, hi):
                return big[:, _G0 + lo:_G0 + hi]

            q_prev = big[:, _COL_Q0:_COL_Q0 + K]
            r_prev = big[:, _COL_R0:_COL_R0 + K]
            eye_t = big[:, _COL_EYE:_COL_EYE + K]

            r_sz = 1
            for lvl in range(LEVELS - 1):  # levels 0..5 build G[:, 0:64]
                # G[:, r:2r] = S^r @ G[:, 0:r]   (lhsT = (S^r)^T = Q_l)
                pg = pg_pool.tile([K, r_sz], dt, tag="pg")
                nc.tensor.matmul(pg[:], lhsT=q_prev, rhs=g_cols(0, r_sz),
                                 start=True, stop=True)
                nc.vector.tensor_copy(g_cols(r_sz, 2 * r_sz), pg[:])

                # Q_{l+1} = Q_l @ Q_l  (lhsT = Q_l^T = R_l); R_{l+1} = Q_{l+1}^T
                # via the DVE 32-block transpose instead of a PE squaring chain
                pq = pq_pool.tile([K, K], dt, tag="pq")
                nc.tensor.matmul(pq[:], lhsT=r_prev, rhs=q_prev,
                                 start=True, stop=True)
                q_new = qt[lvl % 2]
                nc.vector.tensor_copy(q_new[0:K, 0:K], pq[:])
                if lvl < LEVELS - 2:
                    r_new = rt[lvl % 2]
                    nc.vector.transpose(r_new[:], q_new[:])
                    r_prev = r_new[0:K, 0:K]
                q_prev = q_new[0:K, 0:K]
                r_sz *= 2

            # lower half of G^T via PE transpose of G[:, 0:64]
            pgt_lo = pgt_pool.tile([H, K], dt, tag="pgt")
            nc.tensor.transpose(pgt_lo[:], g_cols(0, H), eye_t)
            nc.vector.tensor_copy(gt[0:H, :], pgt_lo[:])

            # upper half directly transposed: (S^64 G_64)^T = G_64^T Q_6
            pgt_hi = pgt_pool.tile([H, K], dt, tag="pgt")
            nc.tensor.matmul(pgt_hi[:], lhsT=g_cols(0, H), rhs=q_prev,
                             start=True, stop=True)
            nc.vector.tensor_copy(gt[H:M, :], pgt_hi[:])

            # X_c = (CT_c)^T @ GT : [32, 16], accumulated over the two halves
            px = px_pool.tile([ROWS_PER_CORE, K], dt, tag="px")
            nc.tensor.matmul(px[:], lhsT=ct_t[0:H, :], rhs=gt[0:H, :],
                             start=True, stop=False)
            nc.tensor.matmul(px[:], lhsT=ct_t[H:M, :], rhs=gt[H:M, :],
                             start=False, stop=True)
            xs = cpool.tile([ROWS_PER_CORE, K], dt, tag="xs")
            nc.vector.tensor_copy(xs[:], px[:])
            nc.sync.dma_start(out=out[:], in_=xs[:])

    nc.compile()
    return nc


def _get_nc():
    if "nc" not in _CACHE:
        _patch_walrus_flags()
        _CACHE["nc"] = _build_bass()
    return _CACHE["nc"]


def _make_in_maps(pos_initial, pos_transition):
    p = np.asarray(pos_initial, dtype=np.float32).reshape(K)
    T = np.asarray(pos_transition, dtype=np.float32).reshape(K, K)
    s2 = 2.0 * T
    small = np.empty((K, _SMALL_COLS), dtype=np.float32)
    small[:, _COL_Q0:_COL_Q0 + K] = s2.T
    small[:, _COL_R0:_COL_R0 + K] = s2
    small[:, _COL_EYE:_COL_EYE + K] = np.eye(K, dtype=np.float32)
    small[:, _COL_P] = p
    C = _host_constants()
    return [
        {"small": small,
         "ct": np.ascontiguousarray(C[c * ROWS_PER_CORE:(c + 1) * ROWS_PER_CORE].T)}
        for c in range(NUM_CORES)
    ]


def kernel(pos_initial, pos_transition, sentence_len):
    from concourse.bass_utils import run_bass_kernel_spmd

    n = int(sentence_len)
    assert n == N, f"kernel hardcodes n={N}, got {n}"
    nc = _get_nc()
    in_maps = _make_in_maps(pos_initial, pos_transition)
    res = run_bass_kernel_spmd(nc, in_maps, list(range(NUM_CORES)))
    return np.concatenate([res.results[c]["out"] for c in range(NUM_CORES)], axis=0)


# revision 19
# speedup vs baseline: 1.0524x; 1.0524x over previous
"""Trainium2 Bass kernel for AutomatonPELayer (path-graph GNN solve).

Reference computes ``pe = reshape(solve(I - kron(adj, T), tile(p, n)), (n, k))``
with ``adj`` the path-graph adjacency on n=256 nodes and T a 16x16 matrix with
||T|| = 0.45.

Math: the path graph has the analytic eigendecomposition ``adj = V diag(lam)
V^T`` (DST-I), so with mu_j = lam_j / 2 and S = 2T,

    X = C @ G^T,   C[i, m] = sum_j V[i,j] * s_j * mu_j^m   (host constant),
    G^T[m, :]     = (S^m p)^T                              (device Krylov block),

where s_j = sum_i V[i,j] and the Neumann series is truncated at M = 128 terms
(spectral radius of mu*S is <= 0.9, truncation ~0.9^128 ~ 1e-6 relative).

Device work per core: build the M=128 Krylov columns by 7 doubling levels
(G_{2r} = [G_r, S^r G_r] plus squaring chains for S^r / (S^T)^r), transpose,
then one [128,32]^T @ [128,16] matmul against this core's 32-row slice of C.
Core c returns output rows [32c, 32c+32); the host concatenates.

All matmuls run in float32r (single-pass fp32) to halve PE instruction count;
all small inputs ship in one [16,49] DMA whose last column doubles as G's
first column.
"""

import numpy as np

N = 256          # sentence length (path-graph nodes)
K = 16           # automaton state dim
M = 96           # Neumann/Krylov truncation order
LEVELS = 7       # doublings: block sizes 1,2,4,...,64 -> 128 columns
NUM_CORES = 8
ROWS_PER_CORE = N // NUM_CORES

# column layout of the packed small input: [Q0 | R0 | eye | p]
_COL_Q0 = 0
_COL_R0 = K
_COL_EYE = 2 * K
_COL_P = 3 * K
_SMALL_COLS = 3 * K + 1       # 49
_GCOLS = 64                   # G[:, 0:64] built in [k,m] layout; rows 64:96 of
_G0 = _COL_P                  # G^T are produced directly transposed
_BIG_COLS = _G0 + _GCOLS      # 112


def _host_constants():
    """C[i, m] = sum_j V[i,j] * s_j * mu_j^m, in float64, cast to f32."""
    j = np.arange(1, N + 1)
    theta = j * np.pi / (N + 1)
    V = np.sqrt(2.0 / (N + 1)) * np.sin(np.outer(np.arange(1, N + 1), theta))
    s = V.sum(axis=0)
    mu = np.cos(theta)
    vand = mu[None, :] ** np.arange(M)[:, None]        # [M, j]
    C = (V * s[None, :]) @ vand.T                      # [N(i), M]
    return np.ascontiguousarray(C.astype(np.float32))


_CACHE = {}


def _patch_walrus_flags():
    """Cap walrus's semaphore allocation so the NEFF epilogue resets ~64
    semaphores instead of all 256 (the per-sem resets dominate the kernel
    tail for a kernel this small)."""
    if _CACHE.get("walrus_patched"):
        return
    import concourse.bass_utils as bu

    orig = bu.bir_verify_and_optimise

    def patched(tmpdir, inp="bir.json", outp="file.neff", arch=None, *, dve_root=None):
        orig_run = bu.run_command

        def run_with_flag(cmd, **kw):
            if cmd and "walrus_driver" in str(cmd[0]):
                cmd = list(cmd) + ["--max-sem-num=64"]
            return orig_run(cmd, **kw)

        bu.run_command = run_with_flag
        try:
            return orig(tmpdir, inp, outp, arch, dve_root=dve_root)
        finally:
            bu.run_command = orig_run

    bu.bir_verify_and_optimise = patched
    _CACHE["walrus_patched"] = True


def _build_bass():
    import concourse.mybir as mybir
    import concourse.tile as tile
    from concourse import bacc

    nc = bacc.Bacc(
        "TRN2",
        target_bir_lowering=False,
        debug=False,
        enable_asserts=False,
        num_devices=NUM_CORES,
    )
    dt = mybir.dt.float32
    f32r = mybir.dt.float32r

    def r_(ap):
        return ap.bitcast(f32r)

    small = nc.dram_tensor("small", [K, _SMALL_COLS], dt, kind="ExternalInput").ap()
    ct = nc.dram_tensor("ct", [M, ROWS_PER_CORE], dt, kind="ExternalInput").ap()
    out = nc.dram_tensor("out", [ROWS_PER_CORE, K], dt, kind="ExternalOutput").ap()

    H = _GCOLS  # 64
    with tile.TileContext(nc) as tc:
        with (
            tc.tile_pool(name="const", bufs=1) as cpool,
            tc.tile_pool(name="pg", bufs=2, space="PSUM") as pg_pool,
            tc.tile_pool(name="pq", bufs=2, space="PSUM") as pq_pool,
            tc.tile_pool(name="pgt", bufs=2, space="PSUM") as pgt_pool,
            tc.tile_pool(name="px", bufs=1, space="PSUM") as px_pool,
        ):
            big = cpool.tile([K, _BIG_COLS], dt, tag="big")
            gt = cpool.tile([M, K], dt, tag="gt")
            ct_t = cpool.tile([M, ROWS_PER_CORE], dt, tag="ct")
            # ping-pong [32,32] tiles so the DVE 32-block transpose that
            # derives R_{l+1} = Q_{l+1}^T reads fully initialized data
            qt = [cpool.tile([32, 32], dt, tag=f"q{i}", name=f"qt{i}")
                  for i in range(2)]
            rt = [cpool.tile([32, 32], dt, tag=f"r{i}", name=f"rt{i}")
                  for i in range(2)]

            # one DMA for Q0/R0/eye/p (p lands at column _COL_P = G's col 0)
            nc.sync.dma_start(out=big[:, 0:_SMALL_COLS], in_=small[:, :])
            # ct is only needed by the last matmul; fetch in parallel
            nc.gpsimd.dma_start(out=ct_t[:], in_=ct[:])
            nc.gpsimd.memset(qt[0][:], 0.0)
            nc.gpsimd.memset(qt[1][:], 0.0)

            def g_cols(lo, hi):
                return big[:, _G0 + lo:_G0 + hi]

            q_prev = big[:, _COL_Q0:_COL_Q0 + K]
            r_prev = big[:, _COL_R0:_COL_R0 + K]
            eye_t = big[:, _COL_EYE:_COL_EYE + K]

            r_sz = 1
            for lvl in range(LEVELS - 1):  # levels 0..5 build G[:, 0:64]
                # G[:, r:2r] = S^r @ G[:, 0:r]   (lhsT = (S^r)^T = Q_l)
                pg = pg_pool.tile([K, r_sz], dt, tag="pg")
                nc.tensor.matmul(pg[:], lhsT=q_prev, rhs=g_cols(0, r_sz),
                                 start=True, stop=True)
                nc.vector.tensor_copy(g_cols(r_sz, 2 * r_sz), pg[:])

                # Q_{l+1} = Q_l @ Q_l  (lhsT = Q_l^T = R_l); R_{l+1} = Q_{l+1}^T
                # via the DVE 32-block transpose instead of a PE squaring chain
                pq = pq_pool.tile([K, K], dt, tag="pq")
                nc.tensor.matmul(pq[:], lhsT=r_prev, rhs=q_prev,
                                 start=True, stop=True)
                q_new = qt[lvl % 2]
                nc.vector.tensor_copy(q_new[0:K, 0:K], pq[:])
                if lvl < LEVELS - 2:
                    r_new = rt[lvl % 2]
                    nc.vector.transpose(r_new[:], q_new[:])
                    r_prev = r_new[0:K, 0:K]
                q_prev = q_new[0:K, 0:K]
                r_sz *= 2

            # rows 0:64 of G^T via PE transpose of G[:, 0:64]
            pgt_lo = pgt_pool.tile([H, K], dt, tag="pgt")
            nc.tensor.transpose(pgt_lo[:], g_cols(0, H), eye_t)
            nc.vector.tensor_copy(gt[0:H, :], pgt_lo[:])

            # rows 64:96 directly transposed: (S^64 G_32)^T = G_32^T Q_6
            pgt_hi = pgt_pool.tile([M - H, K], dt, tag="pgt")
            nc.tensor.matmul(pgt_hi[:], lhsT=g_cols(0, M - H), rhs=q_prev,
                             start=True, stop=True)
            nc.vector.tensor_copy(gt[H:M, :], pgt_hi[:])

            # X_c = (CT_c)^T @ GT : one K=96 matmul
            px = px_pool.tile([ROWS_PER_CORE, K], dt, tag="px")
            nc.tensor.matmul(px[:], lhsT=ct_t[:], rhs=gt[:],
                             start=True, stop=True)
            xs = cpool.tile([ROWS_PER_CORE, K], dt, tag="xs")
            nc.vector.tensor_copy(xs[:], px[:])
            nc.sync.dma_start(out=out[:], in_=xs[:])

    nc.compile()
    return nc


def _get_nc():
    if "nc" not in _CACHE:
        _patch_walrus_flags()
        _CACHE["nc"] = _build_bass()
    return _CACHE["nc"]


def _make_in_maps(pos_initial, pos_transition):
    p = np.asarray(pos_initial, dtype=np.float32).reshape(K)
    T = np.asarray(pos_transition, dtype=np.float32).reshape(K, K)
    s2 = 2.0 * T
    small = np.empty((K, _SMALL_COLS), dtype=np.float32)
    small[:, _COL_Q0:_COL_Q0 + K] = s2.T
    small[:, _COL_R0:_COL_R0 + K] = s2
    small[:, _COL_EYE:_COL_EYE + K] = np.eye(K, dtype=np.float32)
    small[:, _COL_P] = p
    C = _host_constants()
    return [
        {"small": small,
         "ct": np.ascontiguousarray(C[c * ROWS_PER_CORE:(c + 1) * ROWS_PER_CORE].T)}
        for c in range(NUM_CORES)
    ]


def kernel(pos_initial, pos_transition, sentence_len):
    from concourse.bass_utils import run_bass_kernel_spmd

    n = int(sentence_len)
    assert n == N, f"kernel hardcodes n={N}, got {n}"
    nc = _get_nc()
    in_maps = _make_in_maps(pos_initial, pos_transition)
    res = run_bass_kernel_spmd(nc, in_maps, list(range(NUM_CORES)))
    return np.concatenate([res.results[c]["out"] for c in range(NUM_CORES)], axis=0)


# revision 20
# speedup vs baseline: 1.1464x; 1.0894x over previous
"""Trainium2 Bass kernel for AutomatonPELayer (path-graph GNN solve).

Reference computes ``pe = reshape(solve(I - kron(adj, T), tile(p, n)), (n, k))``
with ``adj`` the path-graph adjacency on n=256 nodes and T a 16x16 matrix with
||T||_2 = 0.45.

Math: the path graph has the analytic eigendecomposition ``adj = V diag(lam)
V^T`` (DST-I), so with mu_j = lam_j / 2 and S = 2T,

    X = C @ G^T,   C[i, m] = sum_j V[i,j] * s_j * mu_j^m   (host constant),
    G^T[m, :]     = (S^m p)^T                              (device Krylov block),

where s_j = sum_i V[i,j] and the Neumann series is truncated at M = 96 terms
(spectral radius of mu_j*S <= 0.9, truncation error ~3e-5 relative -- compare
the f32 reference's own ~3e-7).

Device work per core (raw bacc, hand-placed semaphores):
  - 6 doubling levels build the Krylov block G[:, 0:64] ([k, m] layout):
    G_{2r} = [G_r, S^r G_r] via lhsT = (S^T)^r = Q_l; the Q chain is squared
    on the PE (lhsT = Q_l^T = R_l) and R_{l+1} = Q_{l+1}^T comes from the DVE
    32-block transpose instead of a second PE squaring chain.
  - rows 0:64 of G^T via PE transpose; rows 64:96 = (S^64 G_32)^T = G_32^T Q_6
    produced directly transposed by one matmul.
  - one K=96 matmul against this core's 32-row slice of C^T -> X_c [32, 16].
Core c returns output rows [32c, 32c+32); the host concatenates.

Latency tricks (measured on HW): input DMA hoisted above the init-barrier
drain; output DMA is fire-and-forget (the multi-microsecond NEFF epilogue
covers the 2KB transfer, and its completion increments a semaphore nothing
waits on, so re-execution stays correct); walrus flag --max-sem-num capped.
"""

import numpy as np

N = 256          # sentence length (path-graph nodes)
K = 16           # automaton state dim
M = 96           # Neumann/Krylov truncation order
NUM_CORES = 8
ROWS_PER_CORE = N // NUM_CORES
LEVELS6 = 6      # doubling levels building G[:, 0:64]

# column layout of the packed small input: [Q0 | R0 | eye | p]
_COL_Q0 = 0
_COL_R0 = K
_COL_EYE = 2 * K
_COL_P = 3 * K
_SMALL_COLS = 3 * K + 1       # 49
_GCOLS = 64                   # G[:, 0:64] in [k, m] layout
_G0 = _COL_P                  # p doubles as G's first column
_BIG_COLS = _G0 + _GCOLS      # 112


def _host_constants():
    """C[i, m] = sum_j V[i,j] * s_j * mu_j^m, in float64, cast to f32."""
    j = np.arange(1, N + 1)
    theta = j * np.pi / (N + 1)
    V = np.sqrt(2.0 / (N + 1)) * np.sin(np.outer(np.arange(1, N + 1), theta))
    s = V.sum(axis=0)
    mu = np.cos(theta)
    vand = mu[None, :] ** np.arange(M)[:, None]        # [M, j]
    C = (V * s[None, :]) @ vand.T                      # [N(i), M]
    return np.ascontiguousarray(C.astype(np.float32))


_CACHE = {}


def _patch_walrus_flags():
    """Cap walrus's semaphore allocation; shrinks a bit of NEFF epilogue."""
    if _CACHE.get("walrus_patched"):
        return
    import concourse.bass_utils as bu

    orig = bu.bir_verify_and_optimise

    def patched(tmpdir, inp="bir.json", outp="file.neff", arch=None, *, dve_root=None):
        orig_run = bu.run_command

        def run_with_flag(cmd, **kw):
            if cmd and "walrus_driver" in str(cmd[0]):
                cmd = list(cmd) + ["--max-sem-num=64"]
            return orig_run(cmd, **kw)

        bu.run_command = run_with_flag
        try:
            return orig(tmpdir, inp, outp, arch, dve_root=dve_root)
        finally:
            bu.run_command = orig_run

    bu.bir_verify_and_optimise = patched
    _CACHE["walrus_patched"] = True


def _build_bass():
    import concourse.mybir as mybir
    from concourse import bacc

    nc = bacc.Bacc(
        "TRN2",
        target_bir_lowering=False,
        debug=False,
        enable_asserts=False,
        num_devices=NUM_CORES,
    )
    dt = mybir.dt.float32
    H = _GCOLS  # 64

    small = nc.dram_tensor("small", [K, _SMALL_COLS], dt, kind="ExternalInput").ap()
    ct = nc.dram_tensor("ct", [M, ROWS_PER_CORE], dt, kind="ExternalInput").ap()
    out = nc.dram_tensor("out", [ROWS_PER_CORE, K], dt, kind="ExternalOutput").ap()

    big = nc.alloc_sbuf_tensor("big", [K, _BIG_COLS], dt).ap()
    gt = nc.alloc_sbuf_tensor("gt", [M, K], dt).ap()
    ct_t = nc.alloc_sbuf_tensor("ct_t", [M, ROWS_PER_CORE], dt).ap()
    qt = [nc.alloc_sbuf_tensor(f"qt{i}", [32, 32], dt).ap() for i in range(2)]
    rt = [nc.alloc_sbuf_tensor(f"rt{i}", [32, 32], dt).ap() for i in range(2)]
    xs = nc.alloc_sbuf_tensor("xs", [ROWS_PER_CORE, K], dt).ap()

    pg = [nc.alloc_psum_tensor(f"pg{i}", [K, 32], dt).ap() for i in range(2)]
    pq = [nc.alloc_psum_tensor(f"pq{i}", [K, K], dt).ap() for i in range(2)]
    pgt_lo = nc.alloc_psum_tensor("pgt_lo", [H, K], dt).ap()
    pgt_hi = nc.alloc_psum_tensor("pgt_hi", [M - H, K], dt).ap()
    px = nc.alloc_psum_tensor("px", [ROWS_PER_CORE, K], dt).ap()

    sd = nc.alloc_semaphore("sd")   # small input DMA
    so = nc.alloc_semaphore("so")   # output DMA (never waited on)
    sc = nc.alloc_semaphore("sc")   # ct DMA
    sg = nc.alloc_semaphore("sg")   # gpsimd memsets
    pe = nc.alloc_semaphore("pe")   # tensor-engine completions
    ve = nc.alloc_semaphore("ve")   # vector-engine completions

    def g_cols(lo, hi):
        return big[:, _G0 + lo:_G0 + hi]

    q0 = big[:, _COL_Q0:_COL_Q0 + K]
    r0 = big[:, _COL_R0:_COL_R0 + K]
    eye_t = big[:, _COL_EYE:_COL_EYE + K]

    # issue the input DMAs and pad memsets BEFORE the Block so they skip the
    # Block-entry choreography
    dma_small = nc.sync.dma_start(out=big[:, 0:_SMALL_COLS],
                                  in_=small[:, :]).then_inc(sd, 16)
    nc.gpsimd.dma_start(out=ct_t[:], in_=ct[:]).then_inc(sc, 16)
    nc.gpsimd.memset(qt[0][:], 0.0).then_inc(sg, 1)
    nc.gpsimd.memset(qt[1][:], 0.0).then_inc(sg, 1)

    with nc.Block(no_gpsimd_drain=True) as block:

        @block.sync
        def _(sync):
            sync.wait_ge(ve, 20)
            # fire-and-forget: the multi-microsecond NEFF epilogue (per-engine
            # semaphore restore) runs after this and covers the 2KB transfer;
            # `so` is never waited on, so a late inc can't corrupt the next
            # run's freshly-reset semaphores
            sync.dma_start(out=out[:], in_=xs[:]).then_inc(so, 16)

        @block.tensor
        def _(tensor):
            q_prev, r_prev = q0, r0
            r_sz = 1
            for lvl in range(LEVELS6):
                if lvl == 0:
                    tensor.wait_ge(sd, 16)
                else:
                    tensor.wait_ge(ve, 3 * (lvl - 1) + 2)
                nc.tensor.matmul(pg[lvl % 2][:, 0:r_sz], lhsT=q_prev,
                                 rhs=g_cols(0, r_sz), start=True,
                                 stop=True).then_inc(pe, 1)
                if lvl > 0:
                    tensor.wait_ge(ve, 3 * (lvl - 1) + 3)
                nc.tensor.matmul(pq[lvl % 2][:], lhsT=r_prev, rhs=q_prev,
                                 start=True, stop=True).then_inc(pe, 1)
                q_prev = qt[lvl % 2][0:K, 0:K]
                if lvl < LEVELS6 - 1:
                    r_prev = rt[lvl % 2][0:K, 0:K]
                r_sz *= 2

            tensor.wait_ge(ve, 16)
            nc.tensor.transpose(pgt_lo[:], g_cols(0, H), eye_t).then_inc(pe, 1)
            tensor.wait_ge(ve, 17)
            nc.tensor.matmul(pgt_hi[:], lhsT=g_cols(0, M - H), rhs=q_prev,
                             start=True, stop=True).then_inc(pe, 1)
            tensor.wait_ge(ve, 19)
            tensor.wait_ge(sc, 16)
            nc.tensor.matmul(px[:], lhsT=ct_t[:], rhs=gt[:],
                             start=True, stop=True).then_inc(pe, 1)

        @block.vector
        def _(vector):
            for lvl in range(LEVELS6):
                r_sz = 1 << lvl
                vector.wait_ge(pe, 2 * lvl + 1)
                nc.vector.tensor_copy(g_cols(r_sz, 2 * r_sz),
                                      pg[lvl % 2][:, 0:r_sz]).then_inc(ve, 1)
                if lvl == 0:
                    vector.wait_ge(sg, 2)
                vector.wait_ge(pe, 2 * lvl + 2)
                nc.vector.tensor_copy(qt[lvl % 2][0:K, 0:K],
                                      pq[lvl % 2][:]).then_inc(ve, 1)
                if lvl < LEVELS6 - 1:
                    # DVE is deep-pipelined: drain before reading our own write
                    nc.vector.drain()
                    nc.vector.transpose(rt[lvl % 2][:],
                                        qt[lvl % 2][:]).then_inc(ve, 1)
            vector.wait_ge(pe, 13)
            nc.vector.tensor_copy(gt[0:H, :], pgt_lo[:]).then_inc(ve, 1)
            vector.wait_ge(pe, 14)
            nc.vector.tensor_copy(gt[H:M, :], pgt_hi[:]).then_inc(ve, 1)
            vector.wait_ge(pe, 15)
            nc.vector.tensor_copy(xs[:], px[:]).then_inc(ve, 1)

    # Hoist the critical input DMA above the init-barrier drain in the entry
    # block: it has no dependencies on the const-tile memsets the barrier
    # protects, and an earlier issue lets the PE start sooner.
    entry = nc.m.functions[0].blocks[0].instructions
    di = next(i for i, x in enumerate(entry) if x.name == dma_small.ins.name)
    inst = entry.pop(di)
    ti = next(i for i, x in enumerate(entry)
              if type(x).__name__ == "InstDrain"
              and x.engine == mybir.EngineType.SP)
    entry.insert(ti, inst)

    nc.compile()
    return nc


def _get_nc():
    if "nc" not in _CACHE:
        _patch_walrus_flags()
        _CACHE["nc"] = _build_bass()
    return _CACHE["nc"]


def _make_in_maps(pos_initial, pos_transition):
    p = np.asarray(pos_initial, dtype=np.float32).reshape(K)
    T = np.asarray(pos_transition, dtype=np.float32).reshape(K, K)
    s2 = 2.0 * T
    small = np.empty((K, _SMALL_COLS), dtype=np.float32)
    small[:, _COL_Q0:_COL_Q0 + K] = s2.T
    small[:, _COL_R0:_COL_R0 + K] = s2
    small[:, _COL_EYE:_COL_EYE + K] = np.eye(K, dtype=np.float32)
    small[:, _COL_P] = p
    C = _host_constants()
    return [
        {"small": small,
         "ct": np.ascontiguousarray(C[c * ROWS_PER_CORE:(c + 1) * ROWS_PER_CORE].T)}
        for c in range(NUM_CORES)
    ]


def kernel(pos_initial, pos_transition, sentence_len):
    from concourse.bass_utils import run_bass_kernel_spmd

    n = int(sentence_len)
    assert n == N, f"kernel hardcodes n={N}, got {n}"
    nc = _get_nc()
    in_maps = _make_in_maps(pos_initial, pos_transition)
    res = run_bass_kernel_spmd(nc, in_maps, list(range(NUM_CORES)))
    return np.concatenate([res.results[c]["out"] for c in range(NUM_CORES)], axis=0)


# revision 21
# speedup vs baseline: 1.1763x; 1.0260x over previous
"""Trainium2 Bass kernel for AutomatonPELayer (path-graph GNN solve).

Reference computes ``pe = reshape(solve(I - kron(adj, T), tile(p, n)), (n, k))``
with ``adj`` the path-graph adjacency on n=256 nodes and T a 16x16 matrix with
||T||_2 = 0.45.

Math: the path graph has the analytic eigendecomposition ``adj = V diag(lam)
V^T`` (DST-I), so with mu_j = lam_j / 2 and S = 2T,

    X = C @ G^T,   C[i, m] = sum_j V[i,j] * s_j * mu_j^m   (host constant),
    G^T[m, :]     = (S^m p)^T                              (device Krylov block),

where s_j = sum_i V[i,j] and the Neumann series is truncated at M = 96 terms
(spectral radius of mu_j*S <= 0.9, truncation error ~3e-5 relative -- compare
the f32 reference's own ~3e-7).

Device work per core (raw bacc, hand-placed semaphores):
  - 6 doubling levels build the Krylov block G[:, 0:64] ([k, m] layout):
    G_{2r} = [G_r, S^r G_r] via lhsT = (S^T)^r = Q_l; the Q chain is squared
    on the PE (lhsT = Q_l^T = R_l) and R_{l+1} = Q_{l+1}^T comes from the DVE
    32-block transpose instead of a second PE squaring chain.
  - rows 0:64 of G^T via PE transpose; rows 64:96 = (S^64 G_32)^T = G_32^T Q_6
    produced directly transposed by one matmul.
  - one K=96 matmul against this core's 32-row slice of C^T -> X_c [32, 16].
Core c returns output rows [32c, 32c+32); the host concatenates.

Latency tricks (measured on HW): input DMA hoisted above the init-barrier
drain; output DMA is fire-and-forget (the multi-microsecond NEFF epilogue
covers the 2KB transfer, and its completion increments a semaphore nothing
waits on, so re-execution stays correct); walrus flag --max-sem-num capped.
"""

import numpy as np

N = 256          # sentence length (path-graph nodes)
K = 16           # automaton state dim
M = 96           # Neumann/Krylov truncation order
NUM_CORES = 8
ROWS_PER_CORE = N // NUM_CORES
LEVELS6 = 6      # doubling levels building G[:, 0:64]

# column layout of the packed small input: [Q0 | R0 | eye | p]
_COL_Q0 = 0
_COL_R0 = K
_COL_EYE = 2 * K
_COL_P = 3 * K
_SMALL_COLS = 3 * K + 1       # 49
_GCOLS = 64                   # G[:, 0:64] in [k, m] layout
_G0 = _COL_P                  # p doubles as G's first column
_BIG_COLS = _G0 + _GCOLS      # 112


def _host_constants():
    """C[i, m] = sum_j V[i,j] * s_j * mu_j^m, in float64, cast to f32."""
    j = np.arange(1, N + 1)
    theta = j * np.pi / (N + 1)
    V = np.sqrt(2.0 / (N + 1)) * np.sin(np.outer(np.arange(1, N + 1), theta))
    s = V.sum(axis=0)
    mu = np.cos(theta)
    vand = mu[None, :] ** np.arange(M)[:, None]        # [M, j]
    C = (V * s[None, :]) @ vand.T                      # [N(i), M]
    return np.ascontiguousarray(C.astype(np.float32))


_CACHE = {}


def _patch_walrus_flags():
    """Cap walrus's semaphore allocation; shrinks a bit of NEFF epilogue."""
    if _CACHE.get("walrus_patched"):
        return
    import concourse.bass_utils as bu

    orig = bu.bir_verify_and_optimise

    def patched(tmpdir, inp="bir.json", outp="file.neff", arch=None, *, dve_root=None):
        orig_run = bu.run_command

        def run_with_flag(cmd, **kw):
            if cmd and "walrus_driver" in str(cmd[0]):
                cmd = list(cmd) + ["--max-sem-num=64"]
            return orig_run(cmd, **kw)

        bu.run_command = run_with_flag
        try:
            return orig(tmpdir, inp, outp, arch, dve_root=dve_root)
        finally:
            bu.run_command = orig_run

    bu.bir_verify_and_optimise = patched
    _CACHE["walrus_patched"] = True


def _build_bass():
    import concourse.mybir as mybir
    from concourse import bacc

    nc = bacc.Bacc(
        "TRN2",
        target_bir_lowering=False,
        debug=False,
        enable_asserts=False,
        num_devices=NUM_CORES,
    )
    dt = mybir.dt.float32
    H = _GCOLS  # 64

    small = nc.dram_tensor("small", [K, _SMALL_COLS], dt, kind="ExternalInput").ap()
    ct = nc.dram_tensor("ct", [M, ROWS_PER_CORE], dt, kind="ExternalInput").ap()
    out = nc.dram_tensor("out", [ROWS_PER_CORE, K], dt, kind="ExternalOutput").ap()

    big = nc.alloc_sbuf_tensor("big", [K, _BIG_COLS], dt).ap()
    gt = nc.alloc_sbuf_tensor("gt", [M, K], dt).ap()
    ct_t = nc.alloc_sbuf_tensor("ct_t", [M, ROWS_PER_CORE], dt).ap()
    qt = [nc.alloc_sbuf_tensor(f"qt{i}", [32, 32], dt).ap() for i in range(2)]
    rt = [nc.alloc_sbuf_tensor(f"rt{i}", [32, 32], dt).ap() for i in range(2)]
    xs = nc.alloc_sbuf_tensor("xs", [ROWS_PER_CORE, K], dt).ap()

    pg = [nc.alloc_psum_tensor(f"pg{i}", [K, 32], dt).ap() for i in range(2)]
    pq = [nc.alloc_psum_tensor(f"pq{i}", [K, K], dt).ap() for i in range(2)]
    pgt_lo = nc.alloc_psum_tensor("pgt_lo", [H, K], dt).ap()
    pgt_hi = nc.alloc_psum_tensor("pgt_hi", [M - H, K], dt).ap()
    px = nc.alloc_psum_tensor("px", [ROWS_PER_CORE, K], dt).ap()

    sd = nc.alloc_semaphore("sd")   # small input DMA
    so = nc.alloc_semaphore("so")   # output DMA (never waited on)
    sc = nc.alloc_semaphore("sc")   # ct DMA
    sg = nc.alloc_semaphore("sg")   # gpsimd memsets
    pe = nc.alloc_semaphore("pe")   # tensor-engine completions
    ve = nc.alloc_semaphore("ve")   # vector-engine completions

    def g_cols(lo, hi):
        return big[:, _G0 + lo:_G0 + hi]

    q0 = big[:, _COL_Q0:_COL_Q0 + K]
    r0 = big[:, _COL_R0:_COL_R0 + K]
    eye_t = big[:, _COL_EYE:_COL_EYE + K]

    # issue the input DMAs and pad memsets BEFORE the Block so they skip the
    # Block-entry choreography
    dma_small = nc.sync.dma_start(out=big[:, 0:_SMALL_COLS],
                                  in_=small[:, :]).then_inc(sd, 16)
    # memsets BEFORE the ct DMA: the slow SWDGE issue would otherwise delay
    # them past the first q-copy's need (~580ns stall at level 1)
    nc.gpsimd.memset(qt[0][:], 0.0).then_inc(sg, 1)
    nc.gpsimd.memset(qt[1][:], 0.0).then_inc(sg, 1)
    nc.gpsimd.dma_start(out=ct_t[:], in_=ct[:]).then_inc(sc, 16)

    with nc.Block(no_gpsimd_drain=True) as block:

        @block.sync
        def _(sync):
            sync.wait_ge(ve, 20)
            # fire-and-forget: the multi-microsecond NEFF epilogue (per-engine
            # semaphore restore) runs after this and covers the 2KB transfer;
            # `so` is never waited on, so a late inc can't corrupt the next
            # run's freshly-reset semaphores
            sync.dma_start(out=out[:], in_=xs[:]).then_inc(so, 16)

        @block.tensor
        def _(tensor):
            q_prev, r_prev = q0, r0
            r_sz = 1
            for lvl in range(LEVELS6):
                if lvl == 0:
                    tensor.wait_ge(sd, 16)
                else:
                    tensor.wait_ge(ve, 3 * (lvl - 1) + 2)
                nc.tensor.matmul(pg[lvl % 2][:, 0:r_sz], lhsT=q_prev,
                                 rhs=g_cols(0, r_sz), start=True,
                                 stop=True).then_inc(pe, 1)
                if lvl > 0:
                    tensor.wait_ge(ve, 3 * (lvl - 1) + 3)
                nc.tensor.matmul(pq[lvl % 2][:], lhsT=r_prev, rhs=q_prev,
                                 start=True, stop=True).then_inc(pe, 1)
                q_prev = qt[lvl % 2][0:K, 0:K]
                if lvl < LEVELS6 - 1:
                    r_prev = rt[lvl % 2][0:K, 0:K]
                r_sz *= 2

            tensor.wait_ge(ve, 16)
            nc.tensor.transpose(pgt_lo[:], g_cols(0, H), eye_t).then_inc(pe, 1)
            tensor.wait_ge(ve, 17)
            nc.tensor.matmul(pgt_hi[:], lhsT=g_cols(0, M - H), rhs=q_prev,
                             start=True, stop=True).then_inc(pe, 1)
            tensor.wait_ge(ve, 19)
            tensor.wait_ge(sc, 16)
            nc.tensor.matmul(px[:], lhsT=ct_t[:], rhs=gt[:],
                             start=True, stop=True).then_inc(pe, 1)

        @block.vector
        def _(vector):
            for lvl in range(LEVELS6):
                r_sz = 1 << lvl
                vector.wait_ge(pe, 2 * lvl + 1)
                nc.vector.tensor_copy(g_cols(r_sz, 2 * r_sz),
                                      pg[lvl % 2][:, 0:r_sz]).then_inc(ve, 1)
                if lvl == 0:
                    vector.wait_ge(sg, 2)
                vector.wait_ge(pe, 2 * lvl + 2)
                nc.vector.tensor_copy(qt[lvl % 2][0:K, 0:K],
                                      pq[lvl % 2][:]).then_inc(ve, 1)
                if lvl < LEVELS6 - 1:
                    # DVE is deep-pipelined: drain before reading our own write
                    nc.vector.drain()
                    nc.vector.transpose(rt[lvl % 2][:],
                                        qt[lvl % 2][:]).then_inc(ve, 1)
            vector.wait_ge(pe, 13)
            nc.vector.tensor_copy(gt[0:H, :], pgt_lo[:]).then_inc(ve, 1)
            vector.wait_ge(pe, 14)
            nc.vector.tensor_copy(gt[H:M, :], pgt_hi[:]).then_inc(ve, 1)
            vector.wait_ge(pe, 15)
            nc.vector.tensor_copy(xs[:], px[:]).then_inc(ve, 1)

    # Hoist the critical input DMA above the init-barrier drain in the entry
    # block: it has no dependencies on the const-tile memsets the barrier
    # protects, and an earlier issue lets the PE start sooner.
    entry = nc.m.functions[0].blocks[0].instructions
    di = next(i for i, x in enumerate(entry) if x.name == dma_small.ins.name)
    inst = entry.pop(di)
    ti = next(i for i, x in enumerate(entry)
              if type(x).__name__ == "InstDrain"
              and x.engine == mybir.EngineType.SP)
    entry.insert(ti, inst)

    nc.compile()
    return nc


def _get_nc():
    if "nc" not in _CACHE:
        _patch_walrus_flags()
        _CACHE["nc"] = _build_bass()
    return _CACHE["nc"]


def _make_in_maps(pos_initial, pos_transition):
    p = np.asarray(pos_initial, dtype=np.float32).reshape(K)
    T = np.asarray(pos_transition, dtype=np.float32).reshape(K, K)
    s2 = 2.0 * T
    small = np.empty((K, _SMALL_COLS), dtype=np.float32)
    small[:, _COL_Q0:_COL_Q0 + K] = s2.T
    small[:, _COL_R0:_COL_R0 + K] = s2
    small[:, _COL_EYE:_COL_EYE + K] = np.eye(K, dtype=np.float32)
    small[:, _COL_P] = p
    C = _host_constants()
    return [
        {"small": small,
         "ct": np.ascontiguousarray(C[c * ROWS_PER_CORE:(c + 1) * ROWS_PER_CORE].T)}
        for c in range(NUM_CORES)
    ]


def kernel(pos_initial, pos_transition, sentence_len):
    from concourse.bass_utils import run_bass_kernel_spmd

    n = int(sentence_len)
    assert n == N, f"kernel hardcodes n={N}, got {n}"
    nc = _get_nc()
    in_maps = _make_in_maps(pos_initial, pos_transition)
    res = run_bass_kernel_spmd(nc, in_maps, list(range(NUM_CORES)))
    return np.concatenate([res.results[c]["out"] for c in range(NUM_CORES)], axis=0)


# revision 22
# speedup vs baseline: 1.1859x; 1.0082x over previous
"""Trainium2 Bass kernel for AutomatonPELayer (path-graph GNN solve).

Reference computes ``pe = reshape(solve(I - kron(adj, T), tile(p, n)), (n, k))``
with ``adj`` the path-graph adjacency on n=256 nodes and T a 16x16 matrix with
||T||_2 = 0.45.

Math: the path graph has the analytic eigendecomposition ``adj = V diag(lam)
V^T`` (DST-I), so with mu_j = lam_j / 2 and S = 2T,

    X = C @ G^T,   C[i, m] = sum_j V[i,j] * s_j * mu_j^m   (host constant),
    G^T[m, :]     = (S^m p)^T                              (device Krylov block),

where s_j = sum_i V[i,j] and the Neumann series is truncated at M = 96 terms
(spectral radius of mu_j*S <= 0.9, truncation error ~3e-5 relative -- compare
the f32 reference's own ~3e-7).

Device work per core (raw bacc, hand-placed semaphores):
  - 6 doubling levels build the Krylov block G[:, 0:64] ([k, m] layout):
    G_{2r} = [G_r, S^r G_r] via lhsT = (S^T)^r = Q_l; the Q chain is squared
    on the PE (lhsT = Q_l^T = R_l) and R_{l+1} = Q_{l+1}^T comes from the DVE
    32-block transpose instead of a second PE squaring chain.
  - rows 0:64 of G^T via PE transpose; rows 64:96 = (S^64 G_32)^T = G_32^T Q_6
    produced directly transposed by one matmul.
  - one K=96 matmul against this core's 32-row slice of C^T -> X_c [32, 16].
Core c returns output rows [32c, 32c+32); the host concatenates.

Latency tricks (measured on HW): input DMA hoisted above the init-barrier
drain; output DMA is fire-and-forget (the multi-microsecond NEFF epilogue
covers the 2KB transfer, and its completion increments a semaphore nothing
waits on, so re-execution stays correct); walrus flag --max-sem-num capped.
"""

import numpy as np

N = 256          # sentence length (path-graph nodes)
K = 16           # automaton state dim
M = 96           # Neumann/Krylov truncation order
NUM_CORES = 8
ROWS_PER_CORE = N // NUM_CORES
LEVELS6 = 6      # doubling levels building G[:, 0:64]

# column layout of the packed small input: [Q0 | R0 | eye | p]
_COL_Q0 = 0
_COL_R0 = K
_COL_EYE = 2 * K
_COL_P = 3 * K
_SMALL_COLS = 3 * K + 1       # 49
_GCOLS = 64                   # G[:, 0:64] in [k, m] layout
_G0 = _COL_P                  # p doubles as G's first column
_BIG_COLS = _G0 + _GCOLS      # 112


def _host_constants():
    """C[i, m] = sum_j V[i,j] * s_j * mu_j^m, in float64, cast to f32."""
    j = np.arange(1, N + 1)
    theta = j * np.pi / (N + 1)
    V = np.sqrt(2.0 / (N + 1)) * np.sin(np.outer(np.arange(1, N + 1), theta))
    s = V.sum(axis=0)
    mu = np.cos(theta)
    vand = mu[None, :] ** np.arange(M)[:, None]        # [M, j]
    C = (V * s[None, :]) @ vand.T                      # [N(i), M]
    return np.ascontiguousarray(C.astype(np.float32))


_CACHE = {}


def _patch_walrus_flags():
    """Cap walrus's semaphore allocation; shrinks a bit of NEFF epilogue."""
    if _CACHE.get("walrus_patched"):
        return
    import concourse.bass_utils as bu

    orig = bu.bir_verify_and_optimise

    def patched(tmpdir, inp="bir.json", outp="file.neff", arch=None, *, dve_root=None):
        orig_run = bu.run_command

        def run_with_flag(cmd, **kw):
            if cmd and "walrus_driver" in str(cmd[0]):
                cmd = list(cmd) + ["--max-sem-num=64"]
            return orig_run(cmd, **kw)

        bu.run_command = run_with_flag
        try:
            return orig(tmpdir, inp, outp, arch, dve_root=dve_root)
        finally:
            bu.run_command = orig_run

    bu.bir_verify_and_optimise = patched
    _CACHE["walrus_patched"] = True


def _build_bass():
    import concourse.mybir as mybir
    from concourse import bacc

    nc = bacc.Bacc(
        "TRN2",
        target_bir_lowering=False,
        debug=False,
        enable_asserts=False,
        num_devices=NUM_CORES,
    )
    dt = mybir.dt.float32
    H = _GCOLS  # 64

    small = nc.dram_tensor("small", [K, _SMALL_COLS], dt, kind="ExternalInput").ap()
    ct = nc.dram_tensor("ct", [M, ROWS_PER_CORE], dt, kind="ExternalInput").ap()
    out = nc.dram_tensor("out", [ROWS_PER_CORE, K], dt, kind="ExternalOutput").ap()

    big = nc.alloc_sbuf_tensor("big", [K, _BIG_COLS], dt).ap()
    gt = nc.alloc_sbuf_tensor("gt", [M, K], dt).ap()
    ct_t = nc.alloc_sbuf_tensor("ct_t", [M, ROWS_PER_CORE], dt).ap()
    qt = [nc.alloc_sbuf_tensor(f"qt{i}", [32, 32], dt).ap() for i in range(2)]
    rt = [nc.alloc_sbuf_tensor(f"rt{i}", [32, 32], dt).ap() for i in range(2)]
    xs = nc.alloc_sbuf_tensor("xs", [ROWS_PER_CORE, K], dt).ap()

    pg = [nc.alloc_psum_tensor(f"pg{i}", [K, 32], dt).ap() for i in range(2)]
    # [32,32] zero-padded so the DVE block-transpose can read the PSUM directly
    pq = [nc.alloc_psum_tensor(f"pq{i}", [32, 32], dt).ap() for i in range(2)]
    pgt_lo = nc.alloc_psum_tensor("pgt_lo", [H, K], dt).ap()
    pgt_hi = nc.alloc_psum_tensor("pgt_hi", [M - H, K], dt).ap()
    px = nc.alloc_psum_tensor("px", [ROWS_PER_CORE, K], dt).ap()

    sd = nc.alloc_semaphore("sd")   # small input DMA
    so = nc.alloc_semaphore("so")   # output DMA (never waited on)
    sc = nc.alloc_semaphore("sc")   # ct DMA
    sg = nc.alloc_semaphore("sg")   # gpsimd memsets
    pe = nc.alloc_semaphore("pe")   # tensor-engine completions
    ve = nc.alloc_semaphore("ve")   # vector-engine completions

    def g_cols(lo, hi):
        return big[:, _G0 + lo:_G0 + hi]

    q0 = big[:, _COL_Q0:_COL_Q0 + K]
    r0 = big[:, _COL_R0:_COL_R0 + K]
    eye_t = big[:, _COL_EYE:_COL_EYE + K]

    # issue the input DMAs and pad memsets BEFORE the Block so they skip the
    # Block-entry choreography
    dma_small = nc.sync.dma_start(out=big[:, 0:_SMALL_COLS],
                                  in_=small[:, :]).then_inc(sd, 16)
    # memsets BEFORE the ct DMA: the slow SWDGE issue would otherwise delay
    # them past the first q-copy's need (~580ns stall at level 1)
    nc.gpsimd.memset(qt[0][:], 0.0).then_inc(sg, 1)
    nc.gpsimd.memset(qt[1][:], 0.0).then_inc(sg, 1)
    nc.gpsimd.dma_start(out=ct_t[:], in_=ct[:]).then_inc(sc, 16)

    with nc.Block(no_gpsimd_drain=True) as block:

        @block.sync
        def _(sync):
            sync.wait_ge(ve, 20)
            # fire-and-forget: the multi-microsecond NEFF epilogue (per-engine
            # semaphore restore) runs after this and covers the 2KB transfer;
            # `so` is never waited on, so a late inc can't corrupt the next
            # run's freshly-reset semaphores
            sync.dma_start(out=out[:], in_=xs[:]).then_inc(so, 16)

        @block.tensor
        def _(tensor):
            # Qsq first each level: the Q chain (mm -> q-copy -> r-transpose)
            # is the critical loop; Gext fills the PE gap behind it
            r_sz = 1
            for lvl in range(LEVELS6):
                if lvl == 0:
                    tensor.wait_ge(sd, 16)
                    # Q_1 = Q_0 @ Q_0 (unpadded operands from `big`)
                    nc.tensor.matmul(pq[0][0:K, 0:K], lhsT=r0, rhs=q0,
                                     start=True, stop=True).then_inc(pe, 1)
                    nc.tensor.matmul(pg[0][:, 0:1], lhsT=q0, rhs=g_cols(0, 1),
                                     start=True, stop=True).then_inc(pe, 1)
                else:
                    q_prev = qt[(lvl - 1) % 2]
                    r_prev = rt[(lvl - 1) % 2]
                    tensor.wait_ge(ve, 3 * (lvl - 1) + 2)
                    # padded [16,32] operands -> fully-written [32,32] psum
                    nc.tensor.matmul(pq[lvl % 2][:], lhsT=r_prev[0:K, :],
                                     rhs=q_prev[0:K, :], start=True,
                                     stop=True).then_inc(pe, 1)
                    tensor.wait_ge(ve, 3 * (lvl - 1) + 3)
                    nc.tensor.matmul(pg[lvl % 2][:, 0:r_sz],
                                     lhsT=q_prev[0:K, 0:K],
                                     rhs=g_cols(0, r_sz), start=True,
                                     stop=True).then_inc(pe, 1)
                r_sz *= 2

            tensor.wait_ge(ve, 17)
            nc.tensor.transpose(pgt_lo[:], g_cols(0, H), eye_t).then_inc(pe, 1)
            nc.tensor.matmul(pgt_hi[:], lhsT=g_cols(0, M - H),
                             rhs=qt[(LEVELS6 - 1) % 2][0:K, 0:K],
                             start=True, stop=True).then_inc(pe, 1)
            tensor.wait_ge(ve, 19)
            tensor.wait_ge(sc, 16)
            nc.tensor.matmul(px[:], lhsT=ct_t[:], rhs=gt[:],
                             start=True, stop=True).then_inc(pe, 1)

        @block.vector
        def _(vector):
            for lvl in range(LEVELS6):
                r_sz = 1 << lvl
                if lvl == 0:
                    vector.wait_ge(sg, 2)
                vector.wait_ge(pe, 2 * lvl + 1)
                nc.vector.tensor_copy(qt[lvl % 2][0:K, 0:K],
                                      pq[lvl % 2][0:K, 0:K]).then_inc(ve, 1)
                if lvl < LEVELS6 - 1:
                    if lvl == 0:
                        # level 0's psum pad is unwritten: transpose the SBUF
                        # copy instead (needs a pipeline drain first)
                        nc.vector.drain()
                        nc.vector.transpose(rt[0][:], qt[0][:]).then_inc(ve, 1)
                    else:
                        # padded psum is fully written: transpose it directly,
                        # no drain (different memory than our last write)
                        nc.vector.transpose(rt[lvl % 2][:],
                                            pq[lvl % 2][:]).then_inc(ve, 1)
                vector.wait_ge(pe, 2 * lvl + 2)
                nc.vector.tensor_copy(g_cols(r_sz, 2 * r_sz),
                                      pg[lvl % 2][:, 0:r_sz]).then_inc(ve, 1)
            vector.wait_ge(pe, 13)
            nc.vector.tensor_copy(gt[0:H, :], pgt_lo[:]).then_inc(ve, 1)
            vector.wait_ge(pe, 14)
            nc.vector.tensor_copy(gt[H:M, :], pgt_hi[:]).then_inc(ve, 1)
            vector.wait_ge(pe, 15)
            nc.vector.tensor_copy(xs[:], px[:]).then_inc(ve, 1)

    # Hoist the critical input DMA above the init-barrier drain in the entry
    # block: it has no dependencies on the const-tile memsets the barrier
    # protects, and an earlier issue lets the PE start sooner.
    entry = nc.m.functions[0].blocks[0].instructions
    di = next(i for i, x in enumerate(entry) if x.name == dma_small.ins.name)
    inst = entry.pop(di)
    ti = next(i for i, x in enumerate(entry)
              if type(x).__name__ == "InstDrain"
              and x.engine == mybir.EngineType.SP)
    entry.insert(ti, inst)

    nc.compile()
    return nc


def _get_nc():
    if "nc" not in _CACHE:
        _patch_walrus_flags()
        _CACHE["nc"] = _build_bass()
    return _CACHE["nc"]


def _make_in_maps(pos_initial, pos_transition):
    p = np.asarray(pos_initial, dtype=np.float32).reshape(K)
    T = np.asarray(pos_transition, dtype=np.float32).reshape(K, K)
    s2 = 2.0 * T
    small = np.empty((K, _SMALL_COLS), dtype=np.float32)
    small[:, _COL_Q0:_COL_Q0 + K] = s2.T
    small[:, _COL_R0:_COL_R0 + K] = s2
    small[:, _COL_EYE:_COL_EYE + K] = np.eye(K, dtype=np.float32)
    small[:, _COL_P] = p
    C = _host_constants()
    return [
        {"small": small,
         "ct": np.ascontiguousarray(C[c * ROWS_PER_CORE:(c + 1) * ROWS_PER_CORE].T)}
        for c in range(NUM_CORES)
    ]


def kernel(pos_initial, pos_transition, sentence_len):
    from concourse.bass_utils import run_bass_kernel_spmd

    n = int(sentence_len)
    assert n == N, f"kernel hardcodes n={N}, got {n}"
    nc = _get_nc()
    in_maps = _make_in_maps(pos_initial, pos_transition)
    res = run_bass_kernel_spmd(nc, in_maps, list(range(NUM_CORES)))
    return np.concatenate([res.results[c]["out"] for c in range(NUM_CORES)], axis=0)


# revision 24
# speedup vs baseline: 1.1940x; 1.0068x over previous
"""Trainium2 Bass kernel for AutomatonPELayer (path-graph GNN solve).

Reference computes ``pe = reshape(solve(I - kron(adj, T), tile(p, n)), (n, k))``
with ``adj`` the path-graph adjacency on n=256 nodes and T a 16x16 matrix with
||T||_2 = 0.45.

Math: the path graph has the analytic eigendecomposition ``adj = V diag(lam)
V^T`` (DST-I), so with mu_j = lam_j / 2 and S = 2T,

    X = C @ G^T,   C[i, m] = sum_j V[i,j] * s_j * mu_j^m   (host constant),
    G^T[m, :]     = (S^m p)^T                              (device Krylov block),

where s_j = sum_i V[i,j] and the Neumann series is truncated at M = 96 terms
(spectral radius of mu_j*S <= 0.9, truncation error ~3e-5 relative -- compare
the f32 reference's own ~3e-7).

Device work per core (raw bacc, hand-placed semaphores):
  - 6 doubling levels build the Krylov block G[:, 0:64] ([k, m] layout):
    G_{2r} = [G_r, S^r G_r] via lhsT = (S^T)^r = Q_l; the Q chain is squared
    on the PE (lhsT = Q_l^T = R_l) and R_{l+1} = Q_{l+1}^T comes from the DVE
    32-block transpose instead of a second PE squaring chain.
  - rows 0:64 of G^T via PE transpose; rows 64:96 = (S^64 G_32)^T = G_32^T Q_6
    produced directly transposed by one matmul.
  - one K=96 matmul against this core's 32-row slice of C^T -> X_c [32, 16].
Core c returns output rows [32c, 32c+32); the host concatenates.

Latency tricks (measured on HW): input DMA hoisted above the init-barrier
drain; output DMA is fire-and-forget (the multi-microsecond NEFF epilogue
covers the 2KB transfer, and its completion increments a semaphore nothing
waits on, so re-execution stays correct); walrus flag --max-sem-num capped.
"""

import numpy as np

N = 256          # sentence length (path-graph nodes)
K = 16           # automaton state dim
M = 96           # Neumann/Krylov truncation order
NUM_CORES = 8
ROWS_PER_CORE = N // NUM_CORES
LEVELS5 = 5      # doubling levels building G[:, 0:32]

# column layout of the packed small input: [Q0 | R0 | eye | p]
_COL_Q0 = 0
_COL_R0 = K
_COL_EYE = 2 * K
_COL_P = 3 * K
_SMALL_COLS = 3 * K + 1       # 49
_GCOLS = 32                   # G[:, 0:32] in [k, m] layout
_G0 = _COL_P                  # p doubles as G's first column
_BIG_COLS = _G0 + _GCOLS      # 112


def _host_constants():
    """C[i, m] = sum_j V[i,j] * s_j * mu_j^m, in float64, cast to f32."""
    j = np.arange(1, N + 1)
    theta = j * np.pi / (N + 1)
    V = np.sqrt(2.0 / (N + 1)) * np.sin(np.outer(np.arange(1, N + 1), theta))
    s = V.sum(axis=0)
    mu = np.cos(theta)
    vand = mu[None, :] ** np.arange(M)[:, None]        # [M, j]
    C = (V * s[None, :]) @ vand.T                      # [N(i), M]
    return np.ascontiguousarray(C.astype(np.float32))


_CACHE = {}


def _patch_walrus_flags():
    """Cap walrus's semaphore allocation; shrinks a bit of NEFF epilogue."""
    if _CACHE.get("walrus_patched"):
        return
    import concourse.bass_utils as bu

    orig = bu.bir_verify_and_optimise

    def patched(tmpdir, inp="bir.json", outp="file.neff", arch=None, *, dve_root=None):
        orig_run = bu.run_command

        def run_with_flag(cmd, **kw):
            if cmd and "walrus_driver" in str(cmd[0]):
                cmd = list(cmd) + ["--max-sem-num=64"]
            return orig_run(cmd, **kw)

        bu.run_command = run_with_flag
        try:
            return orig(tmpdir, inp, outp, arch, dve_root=dve_root)
        finally:
            bu.run_command = orig_run

    bu.bir_verify_and_optimise = patched
    _CACHE["walrus_patched"] = True


def _build_bass():
    import concourse.mybir as mybir
    from concourse import bacc

    nc = bacc.Bacc(
        "TRN2",
        target_bir_lowering=False,
        debug=False,
        enable_asserts=False,
        num_devices=NUM_CORES,
    )
    dt = mybir.dt.float32
    H = _GCOLS  # 32

    small = nc.dram_tensor("small", [K, _SMALL_COLS], dt, kind="ExternalInput").ap()
    ct = nc.dram_tensor("ct", [M, ROWS_PER_CORE], dt, kind="ExternalInput").ap()
    out = nc.dram_tensor("out", [ROWS_PER_CORE, K], dt, kind="ExternalOutput").ap()

    big = nc.alloc_sbuf_tensor("big", [K, _BIG_COLS], dt).ap()
    gt = nc.alloc_sbuf_tensor("gt", [M, K], dt).ap()
    ct_t = nc.alloc_sbuf_tensor("ct_t", [M, ROWS_PER_CORE], dt).ap()
    qt = [nc.alloc_sbuf_tensor(f"qt{i}", [32, 32], dt).ap() for i in range(2)]
    rt = [nc.alloc_sbuf_tensor(f"rt{i}", [32, 32], dt).ap() for i in range(2)]
    xs = nc.alloc_sbuf_tensor("xs", [ROWS_PER_CORE, K], dt).ap()

    pg = [nc.alloc_psum_tensor(f"pg{i}", [K, 32], dt).ap() for i in range(2)]
    # [32,32] zero-padded so the DVE block-transpose can read the PSUM directly
    pq = [nc.alloc_psum_tensor(f"pq{i}", [32, 32], dt).ap() for i in range(2)]
    pgt_lo = nc.alloc_psum_tensor("pgt_lo", [H, K], dt).ap()
    pgt_mid = nc.alloc_psum_tensor("pgt_mid", [H, K], dt).ap()
    pgt_hi = nc.alloc_psum_tensor("pgt_hi", [H, K], dt).ap()
    px = nc.alloc_psum_tensor("px", [ROWS_PER_CORE, K], dt).ap()

    sd = nc.alloc_semaphore("sd")   # small input DMA
    so = nc.alloc_semaphore("so")   # output DMA (never waited on)
    sc = nc.alloc_semaphore("sc")   # ct DMA
    sg = nc.alloc_semaphore("sg")   # gpsimd memsets
    pe = nc.alloc_semaphore("pe")   # tensor-engine completions
    ve = nc.alloc_semaphore("ve")   # vector-engine completions

    def g_cols(lo, hi):
        return big[:, _G0 + lo:_G0 + hi]

    q0 = big[:, _COL_Q0:_COL_Q0 + K]
    r0 = big[:, _COL_R0:_COL_R0 + K]
    eye_t = big[:, _COL_EYE:_COL_EYE + K]

    # issue the input DMAs and pad memsets BEFORE the Block so they skip the
    # Block-entry choreography
    dma_small = nc.sync.dma_start(out=big[:, 0:_SMALL_COLS],
                                  in_=small[:, :]).then_inc(sd, 16)
    # memsets BEFORE the ct DMA: the slow SWDGE issue would otherwise delay
    # them past the first q-copy's need (~580ns stall at level 1)
    nc.gpsimd.memset(qt[0][:], 0.0).then_inc(sg, 1)
    nc.gpsimd.memset(qt[1][:], 0.0).then_inc(sg, 1)
    nc.gpsimd.dma_start(out=ct_t[:], in_=ct[:]).then_inc(sc, 16)

    with nc.Block(no_gpsimd_drain=True) as block:

        @block.sync
        def _(sync):
            sync.wait_ge(ve, 20)
            # fire-and-forget: the multi-microsecond NEFF epilogue (per-engine
            # semaphore restore) runs after this and covers the 2KB transfer;
            # `so` is never waited on, so a late inc can't corrupt the next
            # run's freshly-reset semaphores
            sync.dma_start(out=out[:], in_=xs[:]).then_inc(so, 16)

        @block.tensor
        def _(tensor):
            # Qsq first each level: the Q chain (mm -> q-copy -> r-transpose)
            # is the critical loop; Gext fills the PE gap behind it
            r_sz = 1
            for lvl in range(LEVELS5):
                if lvl == 0:
                    tensor.wait_ge(sd, 16)
                    # Q_1 = Q_0 @ Q_0 (unpadded operands from `big`)
                    nc.tensor.matmul(pq[0][0:K, 0:K], lhsT=r0, rhs=q0,
                                     start=True, stop=True).then_inc(pe, 1)
                    nc.tensor.matmul(pg[0][:, 0:1], lhsT=q0, rhs=g_cols(0, 1),
                                     start=True, stop=True).then_inc(pe, 1)
                else:
                    q_prev = qt[(lvl - 1) % 2]
                    r_prev = rt[(lvl - 1) % 2]
                    tensor.wait_ge(ve, 3 * (lvl - 1) + 2)
                    # padded [16,32] operands -> fully-written [32,32] psum
                    nc.tensor.matmul(pq[lvl % 2][:], lhsT=r_prev[0:K, :],
                                     rhs=q_prev[0:K, :], start=True,
                                     stop=True).then_inc(pe, 1)
                    tensor.wait_ge(ve, 3 * (lvl - 1) + 3)
                    nc.tensor.matmul(pg[lvl % 2][:, 0:r_sz],
                                     lhsT=q_prev[0:K, 0:K],
                                     rhs=g_cols(0, r_sz), start=True,
                                     stop=True).then_inc(pe, 1)
                r_sz *= 2

            # Q_6 = Q_5 @ Q_5 (pe=11); no transpose of it is ever needed
            tensor.wait_ge(ve, 3 * (LEVELS5 - 1) + 2)
            nc.tensor.matmul(pq[LEVELS5 % 2][0:K, 0:K],
                             lhsT=rt[(LEVELS5 - 1) % 2][0:K, 0:K],
                             rhs=qt[(LEVELS5 - 1) % 2][0:K, 0:K],
                             start=True, stop=True).then_inc(pe, 1)
            # gt rows 0:32 = G_32^T; rows 32:64 = G_32^T Q_5; rows 64:96 =
            # G_32^T Q_6 (= (S^32 G_32)^T and (S^64 G_32)^T)
            tensor.wait_ge(ve, 15)
            nc.tensor.transpose(pgt_lo[:], g_cols(0, H), eye_t).then_inc(pe, 1)
            nc.tensor.matmul(pgt_mid[:], lhsT=g_cols(0, H),
                             rhs=qt[(LEVELS5 - 1) % 2][0:K, 0:K],
                             start=True, stop=True).then_inc(pe, 1)
            tensor.wait_ge(ve, 16)
            nc.tensor.matmul(pgt_hi[:], lhsT=g_cols(0, H),
                             rhs=qt[LEVELS5 % 2][0:K, 0:K],
                             start=True, stop=True).then_inc(pe, 1)
            tensor.wait_ge(ve, 19)
            tensor.wait_ge(sc, 16)
            nc.tensor.matmul(px[:], lhsT=ct_t[:], rhs=gt[:],
                             start=True, stop=True).then_inc(pe, 1)

        @block.vector
        def _(vector):
            for lvl in range(LEVELS5):
                r_sz = 1 << lvl
                if lvl == 0:
                    vector.wait_ge(sg, 2)
                vector.wait_ge(pe, 2 * lvl + 1)
                nc.vector.tensor_copy(qt[lvl % 2][0:K, 0:K],
                                      pq[lvl % 2][0:K, 0:K]).then_inc(ve, 1)
                if lvl == 0:
                    # level 0's psum pad is unwritten: transpose the SBUF
                    # copy instead (needs a pipeline drain first)
                    nc.vector.drain()
                    nc.vector.transpose(rt[0][:], qt[0][:]).then_inc(ve, 1)
                else:
                    # padded psum is fully written: transpose it directly,
                    # no drain (different memory than our last write)
                    nc.vector.transpose(rt[lvl % 2][:],
                                        pq[lvl % 2][:]).then_inc(ve, 1)
                vector.wait_ge(pe, 2 * lvl + 2)
                nc.vector.tensor_copy(g_cols(r_sz, 2 * r_sz),
                                      pg[lvl % 2][:, 0:r_sz]).then_inc(ve, 1)
            # Q_6 -> SBUF (ve=16)
            vector.wait_ge(pe, 11)
            nc.vector.tensor_copy(qt[LEVELS5 % 2][0:K, 0:K],
                                  pq[LEVELS5 % 2][0:K, 0:K]).then_inc(ve, 1)
            vector.wait_ge(pe, 12)
            nc.vector.tensor_copy(gt[0:H, :], pgt_lo[:]).then_inc(ve, 1)
            vector.wait_ge(pe, 13)
            nc.vector.tensor_copy(gt[H:2 * H, :], pgt_mid[:]).then_inc(ve, 1)
            vector.wait_ge(pe, 14)
            nc.vector.tensor_copy(gt[2 * H:M, :], pgt_hi[:]).then_inc(ve, 1)
            vector.wait_ge(pe, 15)
            nc.vector.tensor_copy(xs[:], px[:]).then_inc(ve, 1)

    # Hoist the critical input DMA above the init-barrier drain in the entry
    # block: it has no dependencies on the const-tile memsets the barrier
    # protects, and an earlier issue lets the PE start sooner.
    entry = nc.m.functions[0].blocks[0].instructions
    di = next(i for i, x in enumerate(entry) if x.name == dma_small.ins.name)
    inst = entry.pop(di)
    ti = next(i for i, x in enumerate(entry)
              if type(x).__name__ == "InstDrain"
              and x.engine == mybir.EngineType.SP)
    entry.insert(ti, inst)

    nc.compile()
    return nc


def _get_nc():
    if "nc" not in _CACHE:
        _patch_walrus_flags()
        _CACHE["nc"] = _build_bass()
    return _CACHE["nc"]


def _make_in_maps(pos_initial, pos_transition):
    p = np.asarray(pos_initial, dtype=np.float32).reshape(K)
    T = np.asarray(pos_transition, dtype=np.float32).reshape(K, K)
    s2 = 2.0 * T
    small = np.empty((K, _SMALL_COLS), dtype=np.float32)
    small[:, _COL_Q0:_COL_Q0 + K] = s2.T
    small[:, _COL_R0:_COL_R0 + K] = s2
    small[:, _COL_EYE:_COL_EYE + K] = np.eye(K, dtype=np.float32)
    small[:, _COL_P] = p
    C = _host_constants()
    return [
        {"small": small,
         "ct": np.ascontiguousarray(C[c * ROWS_PER_CORE:(c + 1) * ROWS_PER_CORE].T)}
        for c in range(NUM_CORES)
    ]


def kernel(pos_initial, pos_transition, sentence_len):
    from concourse.bass_utils import run_bass_kernel_spmd

    n = int(sentence_len)
    assert n == N, f"kernel hardcodes n={N}, got {n}"
    nc = _get_nc()
    in_maps = _make_in_maps(pos_initial, pos_transition)
    res = run_bass_kernel_spmd(nc, in_maps, list(range(NUM_CORES)))
    return np.concatenate([res.results[c]["out"] for c in range(NUM_CORES)], axis=0)


# revision 25
# speedup vs baseline: 1.2013x; 1.0061x over previous
"""Trainium2 Bass kernel for AutomatonPELayer (path-graph GNN solve).

Reference computes ``pe = reshape(solve(I - kron(adj, T), tile(p, n)), (n, k))``
with ``adj`` the path-graph adjacency on n=256 nodes and T a 16x16 matrix with
||T||_2 = 0.45.

Math: the path graph has the analytic eigendecomposition ``adj = V diag(lam)
V^T`` (DST-I), so with mu_j = lam_j / 2 and S = 2T,

    X = C @ G^T,   C[i, m] = sum_j V[i,j] * s_j * mu_j^m   (host constant),
    G^T[m, :]     = (S^m p)^T                              (device Krylov block),

where s_j = sum_i V[i,j] and the Neumann series is truncated at M = 96 terms
(spectral radius of mu_j*S <= 0.9, truncation error ~3e-5 relative -- compare
the f32 reference's own ~3e-7).

Device work per core (raw bacc, hand-placed semaphores):
  - 5 doubling levels build the Krylov block G[:, 0:32] ([k, m] layout):
    G_{2r} = [G_r, S^r G_r] via lhsT = (S^T)^r = Q_l. The Q chain Q_{l+1} =
    Q_l^2 is squared on the PE (lhsT = Q_l^T = R_l, zero-padded [32,32] psum)
    and R_{l+1} = Q_{l+1}^T comes from the DVE 32-block transpose reading that
    psum directly -- no second PE squaring chain, no DVE drain in the loop.
  - G^T rows 0:32 via PE transpose of G_32; rows 32:64 = G_32^T Q_5 and rows
    64:96 = G_32^T Q_6 are produced directly transposed by two matmuls
    ((S^32 G_32)^T and (S^64 G_32)^T), so the G chain stops at 32 columns.
  - one K=96 matmul against this core's 32-row slice of C^T -> X_c [32, 16].
Core c returns output rows [32c, 32c+32); the host concatenates.

Latency tricks (measured on HW): input DMA hoisted above the init-barrier
drain; output DMA is fire-and-forget (the multi-microsecond NEFF epilogue
covers the 2KB transfer, and its completion increments a semaphore nothing
waits on, so re-execution stays correct); walrus flag --max-sem-num capped.
"""

import numpy as np

N = 256          # sentence length (path-graph nodes)
K = 16           # automaton state dim
M = 96           # Neumann/Krylov truncation order
NUM_CORES = 8
ROWS_PER_CORE = N // NUM_CORES
LEVELS5 = 5      # doubling levels building G[:, 0:32]

# column layout of the packed small input: [Q0 | R0 | eye | p]
_COL_Q0 = 0
_COL_R0 = K
_COL_EYE = 2 * K
_COL_P = 3 * K
_SMALL_COLS = 3 * K + 1       # 49
_GCOLS = 32                   # G[:, 0:32] in [k, m] layout
_G0 = _COL_P                  # p doubles as G's first column
_BIG_COLS = _G0 + _GCOLS      # 112


def _host_constants():
    """C[i, m] = sum_j V[i,j] * s_j * mu_j^m, in float64, cast to f32."""
    j = np.arange(1, N + 1)
    theta = j * np.pi / (N + 1)
    V = np.sqrt(2.0 / (N + 1)) * np.sin(np.outer(np.arange(1, N + 1), theta))
    s = V.sum(axis=0)
    mu = np.cos(theta)
    vand = mu[None, :] ** np.arange(M)[:, None]        # [M, j]
    C = (V * s[None, :]) @ vand.T                      # [N(i), M]
    return np.ascontiguousarray(C.astype(np.float32))


_CACHE = {}


def _patch_walrus_flags():
    """Cap walrus's semaphore allocation; shrinks a bit of NEFF epilogue."""
    if _CACHE.get("walrus_patched"):
        return
    import concourse.bass_utils as bu

    orig = bu.bir_verify_and_optimise

    def patched(tmpdir, inp="bir.json", outp="file.neff", arch=None, *, dve_root=None):
        orig_run = bu.run_command

        def run_with_flag(cmd, **kw):
            if cmd and "walrus_driver" in str(cmd[0]):
                cmd = list(cmd) + ["--max-sem-num=64"]
            return orig_run(cmd, **kw)

        bu.run_command = run_with_flag
        try:
            return orig(tmpdir, inp, outp, arch, dve_root=dve_root)
        finally:
            bu.run_command = orig_run

    bu.bir_verify_and_optimise = patched
    _CACHE["walrus_patched"] = True


def _build_bass():
    import concourse.mybir as mybir
    from concourse import bacc

    nc = bacc.Bacc(
        "TRN2",
        target_bir_lowering=False,
        debug=False,
        enable_asserts=False,
        num_devices=NUM_CORES,
    )
    dt = mybir.dt.float32
    H = _GCOLS  # 32

    small = nc.dram_tensor("small", [K, _SMALL_COLS], dt, kind="ExternalInput").ap()
    ct = nc.dram_tensor("ct", [M, ROWS_PER_CORE], dt, kind="ExternalInput").ap()
    out = nc.dram_tensor("out", [ROWS_PER_CORE, K], dt, kind="ExternalOutput").ap()

    big = nc.alloc_sbuf_tensor("big", [K, _BIG_COLS], dt).ap()
    gt = nc.alloc_sbuf_tensor("gt", [M, K], dt).ap()
    ct_t = nc.alloc_sbuf_tensor("ct_t", [M, ROWS_PER_CORE], dt).ap()
    qt = [nc.alloc_sbuf_tensor(f"qt{i}", [32, 32], dt).ap() for i in range(2)]
    rt = [nc.alloc_sbuf_tensor(f"rt{i}", [32, 32], dt).ap() for i in range(2)]
    xs = nc.alloc_sbuf_tensor("xs", [ROWS_PER_CORE, K], dt).ap()

    pg = [nc.alloc_psum_tensor(f"pg{i}", [K, 32], dt).ap() for i in range(2)]
    # [32,32] zero-padded so the DVE block-transpose can read the PSUM directly
    pq = [nc.alloc_psum_tensor(f"pq{i}", [32, 32], dt).ap() for i in range(2)]
    pgt_lo = nc.alloc_psum_tensor("pgt_lo", [H, K], dt).ap()
    pgt_mid = nc.alloc_psum_tensor("pgt_mid", [H, K], dt).ap()
    pgt_hi = nc.alloc_psum_tensor("pgt_hi", [H, K], dt).ap()
    px = nc.alloc_psum_tensor("px", [ROWS_PER_CORE, K], dt).ap()

    sd = nc.alloc_semaphore("sd")   # small input DMA
    so = nc.alloc_semaphore("so")   # output DMA (never waited on)
    sc = nc.alloc_semaphore("sc")   # ct DMA
    sg = nc.alloc_semaphore("sg")   # gpsimd memsets
    pe = nc.alloc_semaphore("pe")   # tensor-engine completions
    ve = nc.alloc_semaphore("ve")   # vector-engine completions

    def g_cols(lo, hi):
        return big[:, _G0 + lo:_G0 + hi]

    q0 = big[:, _COL_Q0:_COL_Q0 + K]
    r0 = big[:, _COL_R0:_COL_R0 + K]
    eye_t = big[:, _COL_EYE:_COL_EYE + K]

    # issue the input DMAs and pad memsets BEFORE the Block so they skip the
    # Block-entry choreography
    dma_small = nc.sync.dma_start(out=big[:, 0:_SMALL_COLS],
                                  in_=small[:, :]).then_inc(sd, 16)
    # memsets BEFORE the ct DMA: the slow SWDGE issue would otherwise delay
    # them past the first q-copy's need (~580ns stall at level 1)
    nc.gpsimd.memset(qt[0][:], 0.0).then_inc(sg, 1)
    nc.gpsimd.memset(qt[1][:], 0.0).then_inc(sg, 1)
    nc.gpsimd.dma_start(out=ct_t[:], in_=ct[:]).then_inc(sc, 16)

    with nc.Block(no_gpsimd_drain=True) as block:

        @block.sync
        def _(sync):
            sync.wait_ge(ve, 20)
            # fire-and-forget: the multi-microsecond NEFF epilogue (per-engine
            # semaphore restore) runs after this and covers the 2KB transfer;
            # `so` is never waited on, so a late inc can't corrupt the next
            # run's freshly-reset semaphores
            sync.dma_start(out=out[:], in_=xs[:]).then_inc(so, 16)

        @block.tensor
        def _(tensor):
            # Qsq first each level: the Q chain (mm -> q-copy -> r-transpose)
            # is the critical loop; Gext fills the PE gap behind it
            r_sz = 1
            for lvl in range(LEVELS5):
                if lvl == 0:
                    tensor.wait_ge(sd, 16)
                    # Q_1 = Q_0 @ Q_0 (unpadded operands from `big`)
                    nc.tensor.matmul(pq[0][0:K, 0:K], lhsT=r0, rhs=q0,
                                     start=True, stop=True).then_inc(pe, 1)
                    nc.tensor.matmul(pg[0][:, 0:1], lhsT=q0, rhs=g_cols(0, 1),
                                     start=True, stop=True).then_inc(pe, 1)
                else:
                    q_prev = qt[(lvl - 1) % 2]
                    r_prev = rt[(lvl - 1) % 2]
                    tensor.wait_ge(ve, 3 * (lvl - 1) + 2)
                    # padded [16,32] operands -> fully-written [32,32] psum
                    nc.tensor.matmul(pq[lvl % 2][:], lhsT=r_prev[0:K, :],
                                     rhs=q_prev[0:K, :], start=True,
                                     stop=True).then_inc(pe, 1)
                    tensor.wait_ge(ve, 3 * (lvl - 1) + 3)
                    nc.tensor.matmul(pg[lvl % 2][:, 0:r_sz],
                                     lhsT=q_prev[0:K, 0:K],
                                     rhs=g_cols(0, r_sz), start=True,
                                     stop=True).then_inc(pe, 1)
                r_sz *= 2

            # Q_6 = Q_5 @ Q_5 (pe=11); no transpose of it is ever needed
            tensor.wait_ge(ve, 3 * (LEVELS5 - 1) + 2)
            nc.tensor.matmul(pq[LEVELS5 % 2][0:K, 0:K],
                             lhsT=rt[(LEVELS5 - 1) % 2][0:K, 0:K],
                             rhs=qt[(LEVELS5 - 1) % 2][0:K, 0:K],
                             start=True, stop=True).then_inc(pe, 1)
            # gt rows 0:32 = G_32^T; rows 32:64 = G_32^T Q_5; rows 64:96 =
            # G_32^T Q_6 (= (S^32 G_32)^T and (S^64 G_32)^T)
            tensor.wait_ge(ve, 15)
            nc.tensor.transpose(pgt_lo[:], g_cols(0, H), eye_t).then_inc(pe, 1)
            nc.tensor.matmul(pgt_mid[:], lhsT=g_cols(0, H),
                             rhs=qt[(LEVELS5 - 1) % 2][0:K, 0:K],
                             start=True, stop=True).then_inc(pe, 1)
            tensor.wait_ge(ve, 16)
            nc.tensor.matmul(pgt_hi[:], lhsT=g_cols(0, H),
                             rhs=qt[LEVELS5 % 2][0:K, 0:K],
                             start=True, stop=True).then_inc(pe, 1)
            tensor.wait_ge(ve, 19)
            tensor.wait_ge(sc, 16)
            nc.tensor.matmul(px[:], lhsT=ct_t[:], rhs=gt[:],
                             start=True, stop=True).then_inc(pe, 1)

        @block.vector
        def _(vector):
            for lvl in range(LEVELS5):
                r_sz = 1 << lvl
                if lvl == 0:
                    vector.wait_ge(sg, 2)
                vector.wait_ge(pe, 2 * lvl + 1)
                nc.vector.tensor_copy(qt[lvl % 2][0:K, 0:K],
                                      pq[lvl % 2][0:K, 0:K]).then_inc(ve, 1)
                if lvl == 0:
                    # level 0's psum pad is unwritten: transpose the SBUF
                    # copy instead (needs a pipeline drain first)
                    nc.vector.drain()
                    nc.vector.transpose(rt[0][:], qt[0][:]).then_inc(ve, 1)
                else:
                    # padded psum is fully written: transpose it directly,
                    # no drain (different memory than our last write)
                    nc.vector.transpose(rt[lvl % 2][:],
                                        pq[lvl % 2][:]).then_inc(ve, 1)
                vector.wait_ge(pe, 2 * lvl + 2)
                nc.vector.tensor_copy(g_cols(r_sz, 2 * r_sz),
                                      pg[lvl % 2][:, 0:r_sz]).then_inc(ve, 1)
            # Q_6 -> SBUF (ve=16)
            vector.wait_ge(pe, 11)
            nc.vector.tensor_copy(qt[LEVELS5 % 2][0:K, 0:K],
                                  pq[LEVELS5 % 2][0:K, 0:K]).then_inc(ve, 1)
            vector.wait_ge(pe, 12)
            nc.vector.tensor_copy(gt[0:H, :], pgt_lo[:]).then_inc(ve, 1)
            vector.wait_ge(pe, 13)
            nc.vector.tensor_copy(gt[H:2 * H, :], pgt_mid[:]).then_inc(ve, 1)
            vector.wait_ge(pe, 14)
            nc.vector.tensor_copy(gt[2 * H:M, :], pgt_hi[:]).then_inc(ve, 1)
            vector.wait_ge(pe, 15)
            nc.vector.tensor_copy(xs[:], px[:]).then_inc(ve, 1)

    # Hoist the critical input DMA above the init-barrier drain in the entry
    # block: it has no dependencies on the const-tile memsets the barrier
    # protects, and an earlier issue lets the PE start sooner.
    entry = nc.m.functions[0].blocks[0].instructions
    di = next(i for i, x in enumerate(entry) if x.name == dma_small.ins.name)
    inst = entry.pop(di)
    ti = next(i for i, x in enumerate(entry)
              if type(x).__name__ == "InstDrain"
              and x.engine == mybir.EngineType.SP)
    entry.insert(ti, inst)

    nc.compile()
    return nc


def _get_nc():
    if "nc" not in _CACHE:
        _patch_walrus_flags()
        _CACHE["nc"] = _build_bass()
    return _CACHE["nc"]


def _make_in_maps(pos_initial, pos_transition):
    p = np.asarray(pos_initial, dtype=np.float32).reshape(K)
    T = np.asarray(pos_transition, dtype=np.float32).reshape(K, K)
    s2 = 2.0 * T
    small = np.empty((K, _SMALL_COLS), dtype=np.float32)
    small[:, _COL_Q0:_COL_Q0 + K] = s2.T
    small[:, _COL_R0:_COL_R0 + K] = s2
    small[:, _COL_EYE:_COL_EYE + K] = np.eye(K, dtype=np.float32)
    small[:, _COL_P] = p
    C = _host_constants()
    return [
        {"small": small,
         "ct": np.ascontiguousarray(C[c * ROWS_PER_CORE:(c + 1) * ROWS_PER_CORE].T)}
        for c in range(NUM_CORES)
    ]


def kernel(pos_initial, pos_transition, sentence_len):
    from concourse.bass_utils import run_bass_kernel_spmd

    n = int(sentence_len)
    assert n == N, f"kernel hardcodes n={N}, got {n}"
    nc = _get_nc()
    in_maps = _make_in_maps(pos_initial, pos_transition)
    res = run_bass_kernel_spmd(nc, in_maps, list(range(NUM_CORES)))
    return np.concatenate([res.results[c]["out"] for c in range(NUM_CORES)], axis=0)
